# revision 26
# baseline (speedup 1.0000x reference)
"""Trainium2 Bass kernel for nn_DeMash: y = gather(x)[.,sc] ; out = y @ C^H ; scatter.

Math: out[m,e] = sum_d y[m,d] * conj(C[e,d]),  m in [0,32), d,e in [0,7168).
  out_re = yr@Cr^T + yi@Ci^T ;  out_im = yi@Cr^T - yr@Ci^T

Strategy (8 NeuronCores, memory-bound on the 411MB C read):
  * Column-shard the output dim e across cores (896 cols each) -> each core
    reads its own 1/8 slice of C exactly once. No collectives.
  * Host packs, per core, C^T slices in fp16 hi/lo split form (hi = fp16(C),
    lo = fp16((C-hi)*2048)) so the PE streams at 1 cycle/row (fp32 matmul
    would be 4 cycles/row and PE-bound) while keeping ~fp32 accuracy
    (measured rel err ~5e-7).
  * Stationary operand stacks [yr_hi | yi_hi | s*yr_lo | s*yi_lo] (and the
    [yi | -yr] variant for the Ci stream) so real+imag outputs and the hi/lo
    cross terms all accumulate in PSUM with C streamed exactly once per
    precision half.
  * All DRAM inputs are pre-swizzled on the host into the exact SBUF layout
    (partition-major), so every dma_start is a plain 2D contiguous transfer.
  * Epilogue: out = hi*hi + (lo-terms)/2048, combined on DVE, DMA'd out.

kernel(**inputs) takes FULL inputs, shards on host, runs SPMD on cores 0-7,
and reassembles the FULL [2,B,R,A,S,F] output (gather/scatter on host: 1.8MB
of x traffic vs 411MB of C - negligible, and part of shard/unshard).
"""

import numpy as np
from concurrent.futures import ThreadPoolExecutor

# Problem shape constants (hardcoded: kernel.py must be self-contained).
B, R, A, S, F, NSC = 2, 1, 16, 14, 1024, 512
D = S * NSC          # 7168 contraction/output dim
M = B * R * A        # 32 rows of the skinny GEMM
P = 128              # SBUF partitions / K-chunk
NCORES = 8
ESH = D // NCORES    # 896 output cols per core
NCHUNK = D // P      # 56 K-chunks
EB = 448             # e-block: PSUM bank free-dim (2 blocks of 448 = 896)
G = 2                # K-chunks per C super-tile DMA (~1 MB per dma_start)
NSUPER = NCHUNK // G
LO_SCALE = 2048.0    # 2**11: keeps fp16 lo parts in normal range
_INV = 1.0 / LO_SCALE

_NC_CACHE = {}


def _build_bass():
    import concourse.bass as bass
    import concourse.mybir as mybir
    import concourse.tile as tile

    fp16 = mybir.dt.float16
    fp32 = mybir.dt.float32

    nc = bass.Bass()
    # Single input stream, pre-swizzled on host into exact consumption order:
    # per (mat, s) tile = [ y chunks [P, G*P] | C hi/lo data [P, G*2*ESH] ].
    TW = G * P + G * 2 * ESH  # free-dim elems per stream tile
    c_d = nc.dram_tensor("c", [2 * NSUPER, P, TW], fp16, kind="ExternalInput")
    o_d = nc.dram_tensor("o", [2, M, ESH], fp32, kind="ExternalOutput")

    with tile.TileContext(nc) as tc:
        with (
            tc.tile_pool(name="cpool", bufs=10) as cpool,
            tc.tile_pool(name="epool", bufs=1) as epool,
            tc.tile_pool(name="psum", bufs=1, space="PSUM") as pp,
        ):
            p1 = [pp.tile([P, EB], fp32, tag=f"p1e{e}", name=f"p1e{e}") for e in range(2)]
            p2 = [pp.tile([P, EB], fp32, tag=f"p2e{e}", name=f"p2e{e}") for e in range(2)]

            for s in range(NSUPER):
                for mat in range(2):
                    ct = cpool.tile([P, TW], fp16, tag=f"c{mat}", name=f"c{mat}_{s}")
                    if s == 0:
                        # Split the first two tiles' DMAs so the PE can start
                        # on chunk j as soon as its slice lands.
                        cut = G * P + 2 * ESH
                        nc.sync.dma_start(out=ct[:, 0:cut], in_=c_d[s * 2 + mat][:, 0:cut])
                        nc.sync.dma_start(out=ct[:, cut:TW], in_=c_d[s * 2 + mat][:, cut:TW])
                    else:
                        nc.sync.dma_start(out=ct[:], in_=c_d[s * 2 + mat])
                    for j in range(G):
                        w = ct[:, j * P:(j + 1) * P]
                        cbase = G * P
                        first = (s == 0 and mat == 0 and j == 0)
                        last = (s == NSUPER - 1 and mat == 1 and j == G - 1)
                        for e in range(2):
                            nc.tensor.matmul(
                                p1[e][:],
                                w,
                                ct[:, cbase + (2 * j) * ESH + e * EB:cbase + (2 * j) * ESH + (e + 1) * EB],
                                start=first,
                                stop=last,
                            )
                        for e in range(2):
                            nc.tensor.matmul(
                                p2[e][:],
                                w,
                                ct[:, cbase + (2 * j + 1) * ESH + e * EB:cbase + (2 * j + 1) * ESH + (e + 1) * EB],
                                start=first,
                                stop=last,
                            )

            # Epilogue: out[0:32]=re, out[32:64]=im ; rows 64:128 of p1 carry
            # the (scaled) y_lo x C_hi terms; rows 0:64 of p2 carry y_hi x
            # (scaled) C_lo terms. out = p1_hi + (p1_lo + p2_hi)/2048.
            # ACT does the PSUM->SBUF copies (it can read PSUM) in parallel
            # with DVE's add/scale chain, shortening the serial tail.
            ot = epool.tile([64, 2 * EB], fp32, tag="ot", name="ot")
            t_lo = [
                epool.tile([64, EB], fp32, tag=f"tlo{e}", name=f"tlo{e}")
                for e in range(2)
            ]
            for e in range(2):
                nc.scalar.copy(t_lo[e][:], p2[e][0:64, :])
            for e in range(2):
                es = slice(e * EB, (e + 1) * EB)
                t_sum = epool.tile([64, EB], fp32, tag=f"tsum{e}", name=f"tsum{e}")
                nc.vector.tensor_add(t_sum[:], p1[e][64:128, :], t_lo[e][:])
                nc.vector.scalar_tensor_tensor(
                    ot[:, es],
                    t_sum[:],
                    _INV,
                    p1[e][0:64, :],
                    op0=mybir.AluOpType.mult,
                    op1=mybir.AluOpType.add,
                )
            nc.sync.dma_start(out=o_d.rearrange("c m e -> (c m) e"), in_=ot[:])

    _split_dma_waits(nc, mybir)
    nc.finalize()
    return nc


def _split_dma_waits(nc, mybir):
    """The TPB ISA encodes exactly ONE sync-wait per instruction; walrus
    auto-splits excess waits for compute templates but errors ("Too many sync
    wait commands") for the DMA / Drain / NoOp templates. Tile attaches 2
    waits to slot-reuse DMAs and ~10 to the tail drain. Hoist the extras onto
    preceding single-wait NoOps on the same engine queue -- in-order dispatch
    makes this semantically identical."""
    for fn in nc.m.functions:
        for bb in fn.blocks:
            out = []
            for inst in bb.instructions:
                si = inst.sync_info
                if si is not None and len(si.on_wait) > 1:
                    for w in si.on_wait[:-1]:
                        nop = mybir.InstNoOp(
                            name=nc.get_next_instruction_name(),
                            engine=inst.engine,
                            sync_info=mybir.SyncInfo(on_wait=[w], on_update=[]),
                            ins=[],
                            outs=[],
                            bass_nofuse=True,
                        )
                        nc.register_instruction(nop)
                        out.append(nop)
                    si.on_wait = si.on_wait[-1:]
                out.append(inst)
            bb.instructions[:] = out


def _get_nc():
    if "nc" not in _NC_CACHE:
        _NC_CACHE["nc"] = _build_bass()
    return _NC_CACHE["nc"]


def _hi_lo(a32):
    """fp32 array -> (fp16 hi, fp16 lo*2048). a = hi + lo/2048 + O(2^-22)."""
    hi = a32.astype(np.float16)
    lo = (a32 - hi.astype(np.float32))
    lo *= LO_SCALE
    return hi, lo.astype(np.float16)


def _transpose_blocked(c_slice):
    """[ESH, D] fp32 -> contiguous [D, ESH] fp32, cache-blocked."""
    out = np.empty((D, c_slice.shape[0]), np.float32)
    bs = 64
    for i in range(0, c_slice.shape[0], bs):
        out[:, i:i + bs] = c_slice[i:i + bs, :].T
    return out


def _prep_c_core(C_re, C_im, y_sw, c):
    """Per-core stream pack [2*NSUPER, P, G*P + G*2*ESH]: tile (s, mat) =
    [y chunks | C hi/lo], in exact device consumption order."""
    TW = G * P + G * 2 * ESH
    sl = slice(c * ESH, (c + 1) * ESH)
    cpk = np.empty((2 * NSUPER, P, TW), np.float16)
    for mat, Cm in enumerate((C_re, C_im)):
        t = _transpose_blocked(Cm[sl])           # [D, ESH] = [(s g p), e]
        hi, lo = _hi_lo(t)
        # stack -> [s, g, p, 2, e] -> [s, p, g, 2, e] -> [s, p, g*2*e]
        st = np.stack(
            [hi.reshape(NSUPER, G, P, ESH), lo.reshape(NSUPER, G, P, ESH)], axis=3
        )
        cpart = np.transpose(st, (0, 2, 1, 3, 4)).reshape(NSUPER, P, G * 2 * ESH)
        cpk[mat::2, :, 0:G * P] = y_sw[mat]
        cpk[mat::2, :, G * P:] = cpart
    return cpk


def kernel(x_re, x_im, C_re, C_im, sc_ind):
    from concourse.bass_utils import run_bass_kernel_spmd

    x_re = np.asarray(x_re, dtype=np.float32)
    x_im = np.asarray(x_im, dtype=np.float32)
    C_re = np.asarray(C_re, dtype=np.float32)
    C_im = np.asarray(C_im, dtype=np.float32)
    sc_ind = np.asarray(sc_ind)

    # Host gather (1.8MB): y[m,d] with m=(b,r,a), d=(s,n) via sc_ind.
    yr = x_re.reshape(M, S, F)[:, :, sc_ind].reshape(M, D)
    yi = x_im.reshape(M, S, F)[:, :, sc_ind].reshape(M, D)
    yrT = np.ascontiguousarray(yr.T, dtype=np.float32)  # [D, 32]
    yiT = np.ascontiguousarray(yi.T, dtype=np.float32)
    yr_hi, yr_lo = _hi_lo(yrT)
    yi_hi, yi_lo = _hi_lo(yiT)
    w0 = np.concatenate([yr_hi, yi_hi, yr_lo, yi_lo], axis=1)     # Cr stream
    w1 = np.concatenate([yi_hi, -yr_hi, yi_lo, -yr_lo], axis=1)   # Ci stream
    # [2, (s j p), m] -> per-super-tile partition-major [mat, s, p, (j m)]
    y_st = np.stack([w0, w1]).reshape(2, NSUPER, G, P, P)
    y_sw = np.ascontiguousarray(
        np.transpose(y_st, (0, 1, 3, 2, 4)).reshape(2, NSUPER, P, G * P),
        dtype=np.float16,
    )

    with ThreadPoolExecutor(max_workers=NCORES) as ex:
        futs = [ex.submit(_prep_c_core, C_re, C_im, y_sw, c) for c in range(NCORES)]
        c_parts = [f.result() for f in futs]

    in_maps = [{"c": c_parts[c]} for c in range(NCORES)]

    nc = _get_nc()
    res = run_bass_kernel_spmd(nc, in_maps, list(range(NCORES)))
    _NC_CACHE["last_result"] = res

    # Unshard: concat core e-slices, scatter into the F grid.
    o_mat = np.concatenate([r["o"] for r in res.results], axis=2)  # [2, 32, D]
    out = np.zeros((2, B, R, A, S, F), np.float32)
    out[0].reshape(M, S, F)[:, :, sc_ind] = o_mat[0].reshape(M, S, NSC)
    out[1].reshape(M, S, F)[:, :, sc_ind] = o_mat[1].reshape(M, S, NSC)
    return out


# revision 29
# speedup vs baseline: 1.0117x; 1.0117x over previous
"""Trainium2 Bass kernel for nn_DeMash: y = gather(x)[.,sc] ; out = y @ C^H ; scatter.

Math: out[m,e] = sum_d y[m,d] * conj(C[e,d]),  m in [0,32), d,e in [0,7168).
  out_re = yr@Cr^T + yi@Ci^T ;  out_im = yi@Cr^T - yr@Ci^T

Strategy (8 NeuronCores, memory-bound on the 411MB C read):
  * Column-shard the output dim e across cores (896 cols each) -> each core
    reads its own 1/8 slice of C exactly once. No collectives.
  * Host packs, per core, C^T slices in fp16 hi/lo split form (hi = fp16(C),
    lo = fp16((C-hi)*2048)) so the PE streams at 1 cycle/row (fp32 matmul
    would be 4 cycles/row and PE-bound) while keeping ~fp32 accuracy
    (measured rel err ~5e-7).
  * Stationary operand stacks [yr_hi | yi_hi | s*yr_lo | s*yi_lo] (and the
    [yi | -yr] variant for the Ci stream) so real+imag outputs and the hi/lo
    cross terms all accumulate in PSUM with C streamed exactly once per
    precision half.
  * All DRAM inputs are pre-swizzled on the host into the exact SBUF layout
    (partition-major), so every dma_start is a plain 2D contiguous transfer.
  * Epilogue: out = hi*hi + (lo-terms)/2048, combined on DVE, DMA'd out.

kernel(**inputs) takes FULL inputs, shards on host, runs SPMD on cores 0-7,
and reassembles the FULL [2,B,R,A,S,F] output (gather/scatter on host: 1.8MB
of x traffic vs 411MB of C - negligible, and part of shard/unshard).
"""

import numpy as np
from concurrent.futures import ThreadPoolExecutor

# Problem shape constants (hardcoded: kernel.py must be self-contained).
B, R, A, S, F, NSC = 2, 1, 16, 14, 1024, 512
D = S * NSC          # 7168 contraction/output dim
M = B * R * A        # 32 rows of the skinny GEMM
P = 128              # SBUF partitions / K-chunk
NCORES = 8
ESH = D // NCORES    # 896 output cols per core
NCHUNK = D // P      # 56 K-chunks
EB = 448             # e-block: PSUM bank free-dim (2 blocks of 448 = 896)
G = 2                # K-chunks per C super-tile DMA (~1 MB per dma_start)
NSUPER = NCHUNK // G
LO_SCALE = 2048.0    # 2**11: keeps fp16 lo parts in normal range
_INV = 1.0 / LO_SCALE

_NC_CACHE = {}


def _build_bass():
    import concourse.bass as bass
    import concourse.mybir as mybir
    import concourse.tile as tile

    fp16 = mybir.dt.float16
    fp32 = mybir.dt.float32

    nc = bass.Bass()
    # Single input stream, pre-swizzled on host into exact consumption order:
    # per (mat, s) tile = [ y chunks [P, G*P] | C hi/lo data [P, G*2*ESH] ].
    TW = G * P + G * 2 * ESH  # free-dim elems per stream tile
    c_d = nc.dram_tensor("c", [2 * NSUPER, P, TW], fp16, kind="ExternalInput")
    o_d = nc.dram_tensor("o", [2, M, ESH], fp32, kind="ExternalOutput")

    with tile.TileContext(nc) as tc:
        with (
            tc.tile_pool(name="cpool", bufs=8) as cpool,
            tc.tile_pool(name="epool", bufs=1) as epool,
            tc.tile_pool(name="psum", bufs=1, space="PSUM") as pp,
        ):
            p1 = [pp.tile([P, EB], fp32, tag=f"p1e{e}", name=f"p1e{e}") for e in range(2)]
            p2 = [pp.tile([P, EB], fp32, tag=f"p2e{e}", name=f"p2e{e}") for e in range(2)]

            for s in range(NSUPER):
                for mat in range(2):
                    ct = cpool.tile([P, TW], fp16, tag=f"c{mat}", name=f"c{mat}_{s}")
                    if s == 0:
                        # Split the first two tiles' DMAs so the PE can start
                        # on chunk j as soon as its slice lands.
                        cut = G * P + 2 * ESH
                        nc.sync.dma_start(out=ct[:, 0:cut], in_=c_d[s * 2 + mat][:, 0:cut])
                        nc.sync.dma_start(out=ct[:, cut:TW], in_=c_d[s * 2 + mat][:, cut:TW])
                    else:
                        nc.sync.dma_start(out=ct[:], in_=c_d[s * 2 + mat])
                    for j in range(G):
                        w = ct[:, j * P:(j + 1) * P]
                        cbase = G * P
                        first = (s == 0 and mat == 0 and j == 0)
                        last = (s == NSUPER - 1 and mat == 1 and j == G - 1)
                        for e in range(2):
                            nc.tensor.matmul(
                                p1[e][:],
                                w,
                                ct[:, cbase + (2 * j) * ESH + e * EB:cbase + (2 * j) * ESH + (e + 1) * EB],
                                start=first,
                                stop=last,
                            )
                        for e in range(2):
                            nc.tensor.matmul(
                                p2[e][:],
                                w,
                                ct[:, cbase + (2 * j + 1) * ESH + e * EB:cbase + (2 * j + 1) * ESH + (e + 1) * EB],
                                start=first,
                                stop=last,
                            )

            # Epilogue: out[0:32]=re, out[32:64]=im ; rows 64:128 of p1 carry
            # the (scaled) y_lo x C_hi terms; rows 0:64 of p2 carry y_hi x
            # (scaled) C_lo terms. out = p1_hi + (p1_lo + p2_hi)/2048.
            # ACT does the PSUM->SBUF copies (it can read PSUM) in parallel
            # with DVE's add/scale chain, shortening the serial tail.
            ot = epool.tile([64, 2 * EB], fp32, tag="ot", name="ot")
            t_lo = [
                epool.tile([64, EB], fp32, tag=f"tlo{e}", name=f"tlo{e}")
                for e in range(2)
            ]
            for e in range(2):
                nc.scalar.copy(t_lo[e][:], p2[e][0:64, :])
            for e in range(2):
                es = slice(e * EB, (e + 1) * EB)
                t_sum = epool.tile([64, EB], fp32, tag=f"tsum{e}", name=f"tsum{e}")
                nc.vector.tensor_add(t_sum[:], p1[e][64:128, :], t_lo[e][:])
                nc.vector.scalar_tensor_tensor(
                    ot[:, es],
                    t_sum[:],
                    _INV,
                    p1[e][0:64, :],
                    op0=mybir.AluOpType.mult,
                    op1=mybir.AluOpType.add,
                )
            nc.sync.dma_start(out=o_d.rearrange("c m e -> (c m) e"), in_=ot[:])

    _split_dma_waits(nc, mybir)
    nc.finalize()
    return nc


def _split_dma_waits(nc, mybir):
    """The TPB ISA encodes exactly ONE sync-wait per instruction; walrus
    auto-splits excess waits for compute templates but errors ("Too many sync
    wait commands") for the DMA / Drain / NoOp templates. Tile attaches 2
    waits to slot-reuse DMAs and ~10 to the tail drain. Hoist the extras onto
    preceding single-wait NoOps on the same engine queue -- in-order dispatch
    makes this semantically identical."""
    kinds = (
        mybir.InstDMACopy,
        mybir.InstDrain,
        mybir.InstNoOp,
        mybir.InstTensorTensor,
        mybir.InstTensorScalarPtr,
        mybir.InstTensorCopy,
    )
    for fn in nc.m.functions:
        for bb in fn.blocks:
            out = []
            for inst in bb.instructions:
                si = inst.sync_info
                if isinstance(inst, kinds) and si is not None and len(si.on_wait) > 1:
                    for w in si.on_wait[:-1]:
                        nop = mybir.InstNoOp(
                            name=nc.get_next_instruction_name(),
                            engine=inst.engine,
                            sync_info=mybir.SyncInfo(on_wait=[w], on_update=[]),
                            ins=[],
                            outs=[],
                            bass_nofuse=True,
                        )
                        nc.register_instruction(nop)
                        out.append(nop)
                    si.on_wait = si.on_wait[-1:]
                out.append(inst)
            bb.instructions[:] = out


def _get_nc():
    if "nc" not in _NC_CACHE:
        _NC_CACHE["nc"] = _build_bass()
    return _NC_CACHE["nc"]


def _hi_lo(a32):
    """fp32 array -> (fp16 hi, fp16 lo*2048). a = hi + lo/2048 + O(2^-22)."""
    hi = a32.astype(np.float16)
    lo = (a32 - hi.astype(np.float32))
    lo *= LO_SCALE
    return hi, lo.astype(np.float16)


def _transpose_blocked(c_slice):
    """[ESH, D] fp32 -> contiguous [D, ESH] fp32, cache-blocked."""
    out = np.empty((D, c_slice.shape[0]), np.float32)
    bs = 64
    for i in range(0, c_slice.shape[0], bs):
        out[:, i:i + bs] = c_slice[i:i + bs, :].T
    return out


def _prep_c_core(C_re, C_im, y_sw, c):
    """Per-core stream pack [2*NSUPER, P, G*P + G*2*ESH]: tile (s, mat) =
    [y chunks | C hi/lo], in exact device consumption order."""
    TW = G * P + G * 2 * ESH
    sl = slice(c * ESH, (c + 1) * ESH)
    cpk = np.empty((2 * NSUPER, P, TW), np.float16)
    for mat, Cm in enumerate((C_re, C_im)):
        t = _transpose_blocked(Cm[sl])           # [D, ESH] = [(s g p), e]
        hi, lo = _hi_lo(t)
        # stack -> [s, g, p, 2, e] -> [s, p, g, 2, e] -> [s, p, g*2*e]
        st = np.stack(
            [hi.reshape(NSUPER, G, P, ESH), lo.reshape(NSUPER, G, P, ESH)], axis=3
        )
        cpart = np.transpose(st, (0, 2, 1, 3, 4)).reshape(NSUPER, P, G * 2 * ESH)
        cpk[mat::2, :, 0:G * P] = y_sw[mat]
        cpk[mat::2, :, G * P:] = cpart
    return cpk


def kernel(x_re, x_im, C_re, C_im, sc_ind):
    from concourse.bass_utils import run_bass_kernel_spmd

    x_re = np.asarray(x_re, dtype=np.float32)
    x_im = np.asarray(x_im, dtype=np.float32)
    C_re = np.asarray(C_re, dtype=np.float32)
    C_im = np.asarray(C_im, dtype=np.float32)
    sc_ind = np.asarray(sc_ind)

    # Host gather (1.8MB): y[m,d] with m=(b,r,a), d=(s,n) via sc_ind.
    yr = x_re.reshape(M, S, F)[:, :, sc_ind].reshape(M, D)
    yi = x_im.reshape(M, S, F)[:, :, sc_ind].reshape(M, D)
    yrT = np.ascontiguousarray(yr.T, dtype=np.float32)  # [D, 32]
    yiT = np.ascontiguousarray(yi.T, dtype=np.float32)
    yr_hi, yr_lo = _hi_lo(yrT)
    yi_hi, yi_lo = _hi_lo(yiT)
    w0 = np.concatenate([yr_hi, yi_hi, yr_lo, yi_lo], axis=1)     # Cr stream
    w1 = np.concatenate([yi_hi, -yr_hi, yi_lo, -yr_lo], axis=1)   # Ci stream
    # [2, (s j p), m] -> per-super-tile partition-major [mat, s, p, (j m)]
    y_st = np.stack([w0, w1]).reshape(2, NSUPER, G, P, P)
    y_sw = np.ascontiguousarray(
        np.transpose(y_st, (0, 1, 3, 2, 4)).reshape(2, NSUPER, P, G * P),
        dtype=np.float16,
    )

    with ThreadPoolExecutor(max_workers=NCORES) as ex:
        futs = [ex.submit(_prep_c_core, C_re, C_im, y_sw, c) for c in range(NCORES)]
        c_parts = [f.result() for f in futs]

    in_maps = [{"c": c_parts[c]} for c in range(NCORES)]

    nc = _get_nc()
    last_exc = None
    for _attempt in range(3):
        try:
            res = run_bass_kernel_spmd(nc, in_maps, list(range(NCORES)))
            break
        except Exception as exc:  # transient NRT device faults: retry
            last_exc = exc
            import time
            time.sleep(5)
    else:
        raise last_exc
    _NC_CACHE["last_result"] = res

    # Unshard: concat core e-slices, scatter into the F grid.
    o_mat = np.concatenate([r["o"] for r in res.results], axis=2)  # [2, 32, D]
    out = np.zeros((2, B, R, A, S, F), np.float32)
    out[0].reshape(M, S, F)[:, :, sc_ind] = o_mat[0].reshape(M, S, NSC)
    out[1].reshape(M, S, F)[:, :, sc_ind] = o_mat[1].reshape(M, S, NSC)
    return out


# revision 33
# speedup vs baseline: 1.2029x; 1.1890x over previous
"""Trainium2 Bass kernel for nn_DeMash: y = gather(x)[.,sc] ; out = y @ C^H ; scatter.

Math: out[m,e] = sum_d y[m,d] * conj(C[e,d]),  m in [0,32), d,e in [0,7168).
  out_re = yr@Cr^T + yi@Ci^T ;  out_im = yi@Cr^T - yr@Ci^T

Strategy (8 NeuronCores, memory-bound on the 411MB C read):
  * Column-shard the output dim e across cores (896 cols each) -> each core
    reads its own 1/8 slice of C exactly once. No collectives.
  * Host packs, per core, C^T slices in fp16 hi/lo split form (hi = fp16(C),
    lo = fp16((C-hi)*2048)) so the PE streams at 1 cycle/row (fp32 matmul
    would be 4 cycles/row and PE-bound) while keeping ~fp32 accuracy
    (measured rel err ~5e-7).
  * Stationary operand stacks [yr_hi | yi_hi | s*yr_lo | s*yi_lo] (and the
    [yi | -yr] variant for the Ci stream) so real+imag outputs and the hi/lo
    cross terms all accumulate in PSUM with C streamed exactly once per
    precision half.
  * All DRAM inputs are pre-swizzled on the host into the exact SBUF layout
    (partition-major), so every dma_start is a plain 2D contiguous transfer.
  * Epilogue: out = hi*hi + (lo-terms)/2048, combined on DVE, DMA'd out.

kernel(**inputs) takes FULL inputs, shards on host, runs SPMD on cores 0-7,
and reassembles the FULL [2,B,R,A,S,F] output (gather/scatter on host: 1.8MB
of x traffic vs 411MB of C - negligible, and part of shard/unshard).
"""

import numpy as np
from concurrent.futures import ThreadPoolExecutor

# Problem shape constants (hardcoded: kernel.py must be self-contained).
B, R, A, S, F, NSC = 2, 1, 16, 14, 1024, 512
D = S * NSC          # 7168 contraction/output dim
M = B * R * A        # 32 rows of the skinny GEMM
P = 128              # SBUF partitions / K-chunk
NCORES = 8
ESH = D // NCORES    # 896 output cols per core
NCHUNK = D // P      # 56 K-chunks
EB = 448             # e-block: PSUM bank free-dim (2 blocks of 448 = 896)
G = 2                # K-chunks per C super-tile DMA (~1 MB per dma_start)
NSUPER = NCHUNK // G
LO_SCALE = 2048.0    # 2**11: keeps fp16 lo parts in normal range
_INV = 1.0 / LO_SCALE

_NC_CACHE = {}


def _build_bass():
    import concourse.bass as bass
    import concourse.mybir as mybir
    import concourse.tile as tile

    fp16 = mybir.dt.float16
    fp32 = mybir.dt.float32

    nc = bass.Bass()
    # Single input stream, pre-swizzled on host into exact consumption order.
    # Cr-stream tile: [ W0 y-chunks [P, G*P] | Cr hi/lo data [P, G*2*ESH] ].
    # Ci-stream tile: [ Ci hi/lo data ] only -- its stationary W1 is a signed
    # column permutation of W0, derived on-device by DVE (saves 1.8MB DMA).
    TW0 = G * P + G * 2 * ESH
    TW1 = G * 2 * ESH
    c0_d = nc.dram_tensor("c0", [NSUPER, P, TW0], fp16, kind="ExternalInput")
    c1_d = nc.dram_tensor("c1", [NSUPER, P, TW1], fp16, kind="ExternalInput")
    o_d = nc.dram_tensor("o", [2, M, ESH], fp32, kind="ExternalOutput")

    with tile.TileContext(nc) as tc:
        with (
            tc.tile_pool(name="cpool", bufs=8) as cpool,
            tc.tile_pool(name="epool", bufs=1) as epool,
            tc.tile_pool(name="psum", bufs=1, space="PSUM") as pp,
        ):
            p1 = [pp.tile([P, EB], fp32, tag=f"p1e{e}", name=f"p1e{e}") for e in range(2)]
            p2 = [pp.tile([P, EB], fp32, tag=f"p2e{e}", name=f"p2e{e}") for e in range(2)]

            for s in range(NSUPER):
                for mat in range(2):
                    if mat == 0:
                        ct = cpool.tile([P, TW0], fp16, tag="c0", name=f"c0_{s}")
                        c0t = ct
                        cbase = G * P
                        if s == 0:
                            cut = G * P + 2 * ESH
                            nc.sync.dma_start(out=ct[:, 0:cut], in_=c0_d[s][:, 0:cut])
                            nc.sync.dma_start(out=ct[:, cut:TW0], in_=c0_d[s][:, cut:TW0])
                        else:
                            nc.sync.dma_start(out=ct[:], in_=c0_d[s])
                        # Derive W1 = [yihi|-yrhi|yilo|-yrlo] from W0 per
                        # 64-col half: W1[., 0:32] = W0[., 32:64],
                        # W1[., 32:64] = -W0[., 0:32].
                        w1t = cpool.tile([P, G * P], fp16, tag="w1", name=f"w1_{s}", bufs=4)
                        w1v = w1t.rearrange("p (j h x) -> p j h x", j=G, h=2, x=64)
                        w0v = ct[:, 0:G * P].rearrange("p (j h x) -> p j h x", j=G, h=2, x=64)
                        nc.vector.tensor_copy(w1v[:, :, :, 0:32], w0v[:, :, :, 32:64])
                        nc.vector.tensor_scalar_mul(w1v[:, :, :, 32:64], w0v[:, :, :, 0:32], -1.0)
                    else:
                        ct = cpool.tile([P, TW1], fp16, tag="c1", name=f"c1_{s}")
                        cbase = 0
                        nc.sync.dma_start(out=ct[:], in_=c1_d[s])
                    for j in range(G):
                        if mat == 0:
                            w = c0t[:, j * P:(j + 1) * P]
                        else:
                            w = w1t[:, j * P:(j + 1) * P]
                        first = (s == 0 and mat == 0 and j == 0)
                        last = (s == NSUPER - 1 and mat == 1 and j == G - 1)
                        for e in range(2):
                            nc.tensor.matmul(
                                p1[e][:],
                                w,
                                ct[:, cbase + (2 * j) * ESH + e * EB:cbase + (2 * j) * ESH + (e + 1) * EB],
                                start=first,
                                stop=last,
                            )
                        for e in range(2):
                            nc.tensor.matmul(
                                p2[e][:],
                                w,
                                ct[:, cbase + (2 * j + 1) * ESH + e * EB:cbase + (2 * j + 1) * ESH + (e + 1) * EB],
                                start=first,
                                stop=last,
                            )

            # Epilogue: out[0:32]=re, out[32:64]=im ; rows 64:128 of p1 carry
            # the (scaled) y_lo x C_hi terms; rows 0:64 of p2 carry y_hi x
            # (scaled) C_lo terms. out = p1_hi + (p1_lo + p2_hi)/2048.
            # ACT does the PSUM->SBUF copies (it can read PSUM) in parallel
            # with DVE's add/scale chain, shortening the serial tail.
            ot = epool.tile([64, 2 * EB], fp32, tag="ot", name="ot")
            t_lo = [
                epool.tile([64, EB], fp32, tag=f"tlo{e}", name=f"tlo{e}")
                for e in range(2)
            ]
            for e in range(2):
                nc.scalar.copy(t_lo[e][:], p2[e][0:64, :])
            for e in range(2):
                es = slice(e * EB, (e + 1) * EB)
                t_sum = epool.tile([64, EB], fp32, tag=f"tsum{e}", name=f"tsum{e}")
                nc.vector.tensor_add(t_sum[:], p1[e][64:128, :], t_lo[e][:])
                nc.vector.scalar_tensor_tensor(
                    ot[:, es],
                    t_sum[:],
                    _INV,
                    p1[e][0:64, :],
                    op0=mybir.AluOpType.mult,
                    op1=mybir.AluOpType.add,
                )
            nc.sync.dma_start(out=o_d.rearrange("c m e -> (c m) e"), in_=ot[:])

    _split_dma_waits(nc, mybir)
    nc.finalize()
    return nc


def _split_dma_waits(nc, mybir):
    """The TPB ISA encodes exactly ONE sync-wait per instruction; walrus
    auto-splits excess waits for compute templates but errors ("Too many sync
    wait commands") for the DMA / Drain / NoOp templates. Tile attaches 2
    waits to slot-reuse DMAs and ~10 to the tail drain. Hoist the extras onto
    preceding single-wait NoOps on the same engine queue -- in-order dispatch
    makes this semantically identical."""
    kinds = (
        mybir.InstDMACopy,
        mybir.InstDrain,
        mybir.InstNoOp,
        mybir.InstTensorTensor,
        mybir.InstTensorScalarPtr,
        mybir.InstTensorCopy,
    )
    for fn in nc.m.functions:
        for bb in fn.blocks:
            out = []
            for inst in bb.instructions:
                si = inst.sync_info
                if isinstance(inst, kinds) and si is not None and len(si.on_wait) > 1:
                    for w in si.on_wait[:-1]:
                        nop = mybir.InstNoOp(
                            name=nc.get_next_instruction_name(),
                            engine=inst.engine,
                            sync_info=mybir.SyncInfo(on_wait=[w], on_update=[]),
                            ins=[],
                            outs=[],
                            bass_nofuse=True,
                        )
                        nc.register_instruction(nop)
                        out.append(nop)
                    si.on_wait = si.on_wait[-1:]
                out.append(inst)
            bb.instructions[:] = out


def _get_nc():
    if "nc" not in _NC_CACHE:
        _NC_CACHE["nc"] = _build_bass()
    return _NC_CACHE["nc"]


def _hi_lo(a32):
    """fp32 array -> (fp16 hi, fp16 lo*2048). a = hi + lo/2048 + O(2^-22)."""
    hi = a32.astype(np.float16)
    lo = (a32 - hi.astype(np.float32))
    lo *= LO_SCALE
    return hi, lo.astype(np.float16)


def _transpose_blocked(c_slice):
    """[ESH, D] fp32 -> contiguous [D, ESH] fp32, cache-blocked."""
    out = np.empty((D, c_slice.shape[0]), np.float32)
    bs = 64
    for i in range(0, c_slice.shape[0], bs):
        out[:, i:i + bs] = c_slice[i:i + bs, :].T
    return out


def _prep_c_core(C_re, C_im, y_sw, c):
    """Per-core stream packs: c0 [NSUPER, P, G*P + G*2*ESH] (W0 + Cr data),
    c1 [NSUPER, P, G*2*ESH] (Ci data), in exact device consumption order."""
    sl = slice(c * ESH, (c + 1) * ESH)
    c0 = np.empty((NSUPER, P, G * P + G * 2 * ESH), np.float16)
    c1 = np.empty((NSUPER, P, G * 2 * ESH), np.float16)
    for mat, Cm in enumerate((C_re, C_im)):
        t = _transpose_blocked(Cm[sl])           # [D, ESH] = [(s g p), e]
        hi, lo = _hi_lo(t)
        # stack -> [s, g, p, 2, e] -> [s, p, g, 2, e] -> [s, p, g*2*e]
        st = np.stack(
            [hi.reshape(NSUPER, G, P, ESH), lo.reshape(NSUPER, G, P, ESH)], axis=3
        )
        cpart = np.transpose(st, (0, 2, 1, 3, 4)).reshape(NSUPER, P, G * 2 * ESH)
        if mat == 0:
            c0[:, :, 0:G * P] = y_sw
            c0[:, :, G * P:] = cpart
        else:
            c1[:] = cpart
    return c0, c1


def kernel(x_re, x_im, C_re, C_im, sc_ind):
    from concourse.bass_utils import run_bass_kernel_spmd

    x_re = np.asarray(x_re, dtype=np.float32)
    x_im = np.asarray(x_im, dtype=np.float32)
    C_re = np.asarray(C_re, dtype=np.float32)
    C_im = np.asarray(C_im, dtype=np.float32)
    sc_ind = np.asarray(sc_ind)

    # Host gather (1.8MB): y[m,d] with m=(b,r,a), d=(s,n) via sc_ind.
    yr = x_re.reshape(M, S, F)[:, :, sc_ind].reshape(M, D)
    yi = x_im.reshape(M, S, F)[:, :, sc_ind].reshape(M, D)
    yrT = np.ascontiguousarray(yr.T, dtype=np.float32)  # [D, 32]
    yiT = np.ascontiguousarray(yi.T, dtype=np.float32)
    yr_hi, yr_lo = _hi_lo(yrT)
    yi_hi, yi_lo = _hi_lo(yiT)
    # W0 = [yr_hi | yi_hi | yr_lo | yi_lo]; W1 is derived on-device.
    w0 = np.concatenate([yr_hi, yi_hi, yr_lo, yi_lo], axis=1)
    # [(s j p), m] -> per-super-tile partition-major [s, p, (j m)]
    y_st = w0.reshape(NSUPER, G, P, P)
    y_sw = np.ascontiguousarray(
        np.transpose(y_st, (0, 2, 1, 3)).reshape(NSUPER, P, G * P),
        dtype=np.float16,
    )

    with ThreadPoolExecutor(max_workers=NCORES) as ex:
        futs = [ex.submit(_prep_c_core, C_re, C_im, y_sw, c) for c in range(NCORES)]
        c_parts = [f.result() for f in futs]

    in_maps = [
        {"c0": c_parts[c][0], "c1": c_parts[c][1]} for c in range(NCORES)
    ]

    nc = _get_nc()
    last_exc = None
    for _attempt in range(3):
        try:
            res = run_bass_kernel_spmd(nc, in_maps, list(range(NCORES)))
            break
        except Exception as exc:  # transient NRT device faults: retry
            last_exc = exc
            import time
            time.sleep(5)
    else:
        raise last_exc
    _NC_CACHE["last_result"] = res

    # Unshard: concat core e-slices, scatter into the F grid.
    o_mat = np.concatenate([r["o"] for r in res.results], axis=2)  # [2, 32, D]
    out = np.zeros((2, B, R, A, S, F), np.float32)
    out[0].reshape(M, S, F)[:, :, sc_ind] = o_mat[0].reshape(M, S, NSC)
    out[1].reshape(M, S, F)[:, :, sc_ind] = o_mat[1].reshape(M, S, NSC)
    return out
